# revision 8
# baseline (speedup 1.0000x reference)
"""Trainium2 Bass kernel for nn_Block_15006615734251 (dense transformer block
with chained sigmoid/softmax attention), SPMD over 8 NeuronCores.

Sharding: query rows split 8 ways (512 rows/core/batch). Each core computes
the full K/V projections (replicated), its own rows of attn_obj (sigmoid
scores written straight to HBM), its own rows of the obj->pose context chain,
and the epilogue (proj/LN2/MLP) for its own rows. The only cross-core
exchange is an AllGather of ctx1 = sigmoid(S_obj) @ V  ([2,3,4096,64] bf16).
"""

import numpy as np

B, N, C, H = 2, 4096, 384, 6
HD = C // H            # 64
HALF = C // 2          # 192
MLP_H = 4 * C          # 1536
EPS = 1e-5
NCORES = 8
OWN = N // NCORES      # 512 query rows per core per batch
NMT = N // 128         # 32 key tiles per batch
SCALE = HD ** -0.5

# Head pairs: each pair occupies one 128-partition tensor (member0 in
# partitions 0:64, member1 in 64:128). Obj heads (0-2) pair with obj heads,
# pose (3-5) with pose, so the sigmoid and exp phases stay homogeneous.
PAIRS = [
    ((0, 0), (0, 1)),  # obj
    ((1, 0), (1, 1)),  # obj
    ((0, 2), (1, 2)),  # obj (cross-batch)
    ((0, 3), (0, 4)),  # pose
    ((1, 3), (1, 4)),  # pose
    ((0, 5), (1, 5)),  # pose
]
OBJ_PAIRS = [0, 1, 2]
POSE_PAIRS = [3, 4, 5]


def build_kernel():
    import contextlib
    import concourse.bass as bass
    import concourse.tile as tile
    from concourse import bacc, mybir
    from concourse.masks import make_identity
    from concourse.tile import add_dep_helper

    fp32 = mybir.dt.float32
    bf16 = mybir.dt.bfloat16
    AF = mybir.ActivationFunctionType
    ALU = mybir.AluOpType

    nc = bacc.Bacc("TRN2", target_bir_lowering=False, debug=False,
                   enable_asserts=True, num_devices=NCORES)

    # ---- I/O ----
    x_full = nc.dram_tensor("x_full", [B * N, C], fp32, kind="ExternalInput")
    x_own = nc.dram_tensor("x_own", [B * OWN, C], fp32, kind="ExternalInput")
    wT = nc.dram_tensor("wT", [C, 3 * C], bf16, kind="ExternalInput")
    qb_q = nc.dram_tensor("qb_q", [128, 6], fp32, kind="ExternalInput")
    qb_k = nc.dram_tensor("qb_k", [128, 6], fp32, kind="ExternalInput")
    vb = nc.dram_tensor("vb", [128, HALF], fp32, kind="ExternalInput")
    pwT01 = nc.dram_tensor("pwT01", [128, C], bf16, kind="ExternalInput")
    pwT2 = nc.dram_tensor("pwT2", [128, C], bf16, kind="ExternalInput")
    pbB = nc.dram_tensor("pbB", [128, C], fp32, kind="ExternalInput")
    fc1wT = nc.dram_tensor("fc1wT", [C, MLP_H], bf16, kind="ExternalInput")
    fc1b = nc.dram_tensor("fc1b", [128, MLP_H // 128], fp32, kind="ExternalInput")
    fc2wT = nc.dram_tensor("fc2wT", [MLP_H, C], bf16, kind="ExternalInput")
    fc2bB = nc.dram_tensor("fc2bB", [128, C], fp32, kind="ExternalInput")

    attn_out = nc.dram_tensor("attn_out", [B, 3, OWN, N], fp32, kind="ExternalOutput")
    y_out = nc.dram_tensor("y_out", [B * OWN, C], fp32, kind="ExternalOutput")

    with tile.TileContext(nc) as tc, contextlib.ExitStack() as es:
        # ---------------- persistent pools ----------------
        singles = es.enter_context(tc.tile_pool(name="singles", bufs=1))
        persist = es.enter_context(tc.tile_pool(name="persist", bufs=1))
        dram = es.enter_context(tc.tile_pool(name="dram", bufs=1, space="DRAM"))

        ident = singles.tile([128, 128], bf16)
        make_identity(nc, ident)
        identf = singles.tile([128, 128], fp32)
        make_identity(nc, identf)

        wT_sb = singles.tile([128, 3, 3 * C], bf16)
        nc.sync.dma_start(wT_sb[:], wT.rearrange("(o p) q -> p o q", p=128))
        qbq_sb = singles.tile([128, 6], fp32)
        nc.sync.dma_start(qbq_sb[:], qb_q[:])
        qbk_sb = singles.tile([128, 6], fp32)
        nc.sync.dma_start(qbk_sb[:], qb_k[:])
        vb_sb = singles.tile([128, HALF], fp32)
        nc.sync.dma_start(vb_sb[:], vb[:])
        eps_sb = singles.tile([128, 1], fp32)
        nc.vector.memset(eps_sb[:], EPS)

        x_own_sb = persist.tile([128, B * OWN // 128, C], fp32)
        nc.sync.dma_start(x_own_sb[:], x_own.rearrange("(o p) c -> p o c", p=128))

        kT = [persist.tile([128, N], bf16, name=f"kT{p}") for p in range(6)]
        qT = [persist.tile([128, OWN], bf16, name=f"qT{p}") for p in range(6)]
        v_sb = [persist.tile([128, NMT, HALF], bf16, name=f"v{b}") for b in range(B)]
        c2nT = [persist.tile([128, OWN], bf16, name=f"c2nT{i}") for i in range(3)]

        # ---------------- P0: LN1 -> h^T, then QKV ----------------
        NT_ALL = B * N // 128
        NT_OWN = B * OWN // 128

        with tc.tile_pool(name="hT_pool", bufs=1) as hT_pool, \
             tc.tile_pool(name="p0", bufs=2) as p0, \
             tc.tile_pool(name="p0ps", bufs=1, space="PSUM") as p0ps:
            hT = hT_pool.tile([128, 3, B * N], bf16)
            hoT = hT_pool.tile([128, 3, B * OWN], bf16)

            def ln_tiles(src_ap, ntiles, dst, group=8):
                for g in range(0, ntiles, group):
                    cnt = min(group, ntiles - g)
                    xg = p0.tile([128, group, C], fp32, tag="xg", bufs=2)
                    nc.sync.dma_start(xg[:, :cnt], src_ap[:, g:g + cnt])
                    st = p0.tile([128, group, 6], fp32, tag="st", bufs=2)
                    mv = p0.tile([128, group, 2], fp32, tag="mv", bufs=2)
                    for j in range(cnt):
                        nc.vector.bn_stats(out=st[:, j], in_=xg[:, j])
                        nc.vector.bn_aggr(out=mv[:, j], in_=st[:, j])
                    rstd = p0.tile([128, group], fp32, tag="rstd", bufs=2)
                    nc.scalar.activation(out=rstd[:, :cnt], in_=mv[:, :cnt, 1],
                                         func=AF.Sqrt, bias=eps_sb[:], scale=1.0)
                    nc.vector.reciprocal(out=rstd[:, :cnt], in_=rstd[:, :cnt])
                    for j in range(cnt):
                        hj = p0.tile([128, C], fp32, tag="hj", bufs=2)
                        nc.vector.tensor_scalar(
                            hj[:], xg[:, j], mv[:, j, 0:1], rstd[:, j:j + 1],
                            ALU.subtract, ALU.mult)
                        for cc in range(3):
                            pt = p0ps.tile([128, 128], fp32, tag="trps", bufs=2,
                                           space="PSUM")
                            nc.tensor.transpose(pt[:], hj[:, cc * 128:(cc + 1) * 128],
                                                identf)
                            nc.vector.tensor_copy(
                                out=dst[:, cc, (g + j) * 128:(g + j + 1) * 128],
                                in_=pt[:])

            ln_tiles(x_full.rearrange("(o p) c -> p o c", p=128), NT_ALL, hT)
            ln_tiles(x_own.rearrange("(o p) c -> p o c", p=128), NT_OWN, hoT)

            def k_col(h):
                return C + h * HD

            def q_col(h):
                return h * HD

            for p, ((bA, hA), (bB_, hB)) in enumerate(PAIRS):
                same_b = (bA == bB_)
                for j in range(N // 512):
                    ps = p0ps.tile([128, 512], fp32, tag="qkps", bufs=4, space="PSUM")
                    for cc in range(3):
                        if same_b:
                            nc.tensor.matmul(
                                ps[:], lhsT=wT_sb[:, cc, k_col(hA):k_col(hA) + 128],
                                rhs=hT[:, cc, bA * N + j * 512: bA * N + (j + 1) * 512],
                                start=(cc == 0), stop=(cc == 2))
                        else:
                            nc.tensor.matmul(
                                ps[0:64], lhsT=wT_sb[:, cc, k_col(hA):k_col(hA) + 64],
                                rhs=hT[:, cc, bA * N + j * 512: bA * N + (j + 1) * 512],
                                start=(cc == 0), stop=(cc == 2), tile_position=(0, 0))
                            nc.tensor.matmul(
                                ps[64:128], lhsT=wT_sb[:, cc, k_col(hB):k_col(hB) + 64],
                                rhs=hT[:, cc, bB_ * N + j * 512: bB_ * N + (j + 1) * 512],
                                start=(cc == 0), stop=(cc == 2), tile_position=(0, 64))
                    nc.vector.tensor_scalar(
                        kT[p][:, j * 512:(j + 1) * 512], ps[:],
                        qbk_sb[:, p:p + 1], None, ALU.add)
                ps = p0ps.tile([128, 512], fp32, tag="qkps", bufs=4, space="PSUM")
                for cc in range(3):
                    if same_b:
                        nc.tensor.matmul(
                            ps[:], lhsT=wT_sb[:, cc, q_col(hA):q_col(hA) + 128],
                            rhs=hoT[:, cc, bA * OWN:(bA + 1) * OWN],
                            start=(cc == 0), stop=(cc == 2))
                    else:
                        nc.tensor.matmul(
                            ps[0:64], lhsT=wT_sb[:, cc, q_col(hA):q_col(hA) + 64],
                            rhs=hoT[:, cc, bA * OWN:(bA + 1) * OWN],
                            start=(cc == 0), stop=(cc == 2), tile_position=(0, 0))
                        nc.tensor.matmul(
                            ps[64:128], lhsT=wT_sb[:, cc, q_col(hB):q_col(hB) + 64],
                            rhs=hoT[:, cc, bB_ * OWN:(bB_ + 1) * OWN],
                            start=(cc == 0), stop=(cc == 2), tile_position=(0, 64))
                nc.vector.tensor_scalar(
                    qT[p][:], ps[:], qbq_sb[:, p:p + 1], None, ALU.add)

            for b in range(B):
                for mt in range(NMT):
                    ps = p0ps.tile([128, HALF], fp32, tag="vps", bufs=2, space="PSUM")
                    for cc in range(3):
                        nc.tensor.matmul(
                            ps[:],
                            lhsT=hT[:, cc, b * N + mt * 128: b * N + (mt + 1) * 128],
                            rhs=wT_sb[:, cc, 2 * C + HALF: 3 * C],
                            start=(cc == 0), stop=(cc == 2))
                    nc.vector.tensor_tensor(v_sb[b][:, mt], ps[:], vb_sb[:], ALU.add)

        # ---------------- P1: obj (sigmoid) ----------------
        ctx1_local = dram.tile([B, 3, OWN, HD], bf16)
        ctx1_gath = dram.tile([NCORES, B, 3, OWN, HD], bf16, addr_space="Shared")

        last_sig = None
        with tc.tile_pool(name="p1", bufs=3) as p1, \
             tc.tile_pool(name="p1ps", bufs=1, space="PSUM") as p1ps, \
             tc.tile_pool(name="c1pool", bufs=1) as c1pool:
            ctx1row = c1pool.tile([128, B, 3, OWN // 128, HD], bf16)
            for p in OBJ_PAIRS:
                (bA, hA), (bB_, hB) = PAIRS[p]
                iA, iB = hA, hB
                # col-major S^T -> sigmoid -> ctx1^T accumulation
                ps_c1 = p1ps.tile([128, 512], fp32, tag="c1ps", bufs=1, space="PSUM")
                for mt in range(NMT):
                    ps = p1ps.tile([128, 2 * OWN], fp32, tag="sc", bufs=3, space="PSUM")
                    nc.tensor.matmul(
                        ps[:, 0:OWN], lhsT=kT[p][0:64, mt * 128:(mt + 1) * 128],
                        rhs=qT[p][0:64], tile_position=(0, 0))
                    nc.tensor.matmul(
                        ps[:, OWN:], lhsT=kT[p][64:128, mt * 128:(mt + 1) * 128],
                        rhs=qT[p][64:128], tile_position=(64, 0))
                    at2 = p1.tile([128, 2 * OWN], bf16, tag="at2")
                    last_sig = nc.scalar.activation(out=at2[:], in_=ps[:],
                                                    func=AF.Sigmoid)
                    nc.tensor.matmul(
                        ps_c1[0:64], lhsT=v_sb[bA][:, mt, iA * HD:(iA + 1) * HD],
                        rhs=at2[:, 0:OWN], start=(mt == 0), stop=(mt == NMT - 1),
                        tile_position=(0, 0), skip_group_check=True)
                    nc.tensor.matmul(
                        ps_c1[64:128], lhsT=v_sb[bB_][:, mt, iB * HD:(iB + 1) * HD],
                        rhs=at2[:, OWN:], start=(mt == 0), stop=(mt == NMT - 1),
                        tile_position=(0, 64), skip_group_check=True)
                c1sb = p1.tile([128, 512], bf16, tag="c1sb")
                nc.vector.tensor_copy(out=c1sb[:], in_=ps_c1[:])
                for qc in range(OWN // 128):
                    pt = p1ps.tile([128, 128], bf16, tag="c1tr", bufs=1, space="PSUM")
                    nc.tensor.transpose(pt[:], c1sb[:, qc * 128:(qc + 1) * 128], ident)
                    nc.vector.tensor_copy(out=ctx1row[:, bA, iA, qc], in_=pt[:, 0:64])
                    nc.vector.tensor_copy(out=ctx1row[:, bB_, iB, qc], in_=pt[:, 64:128])

                # row-major S -> sigmoid -> attn_obj slice
                for half in range(2):
                    base = 64 * half
                    b_, i_ = (bA, iA) if half == 0 else (bB_, iB)
                    for nt in range(OWN // 128):
                        for mh in range(N // 1024):
                            ps = p1ps.tile([128, 2 * OWN], fp32, tag="sc", bufs=3,
                                           space="PSUM")
                            for mc in range(2):
                                nc.tensor.matmul(
                                    ps[:, mc * 512:(mc + 1) * 512],
                                    lhsT=qT[p][base:base + 64, nt * 128:(nt + 1) * 128],
                                    rhs=kT[p][base:base + 64,
                                              mh * 1024 + mc * 512:
                                              mh * 1024 + (mc + 1) * 512],
                                    tile_position=(base, 0))
                            stg = p1.tile([128, 1024], fp32, tag="stg")
                            last_sig = nc.scalar.activation(out=stg[:], in_=ps[:],
                                                            func=AF.Sigmoid)
                            nc.sync.dma_start(
                                attn_out[b_, i_, nt * 128:(nt + 1) * 128,
                                         mh * 1024:(mh + 1) * 1024],
                                stg[:])

            nc.sync.dma_start(
                ctx1_local.rearrange("b i (q p) d -> p b i q d", p=128), ctx1row[:])
            nc.gpsimd.collective_compute(
                "AllGather", ALU.bypass,
                replica_groups=[list(range(NCORES))],
                ins=[ctx1_local.opt()], outs=[ctx1_gath.opt()])

        # ---------------- P2: pose (exp softmax) + ctx2 ----------------
        with tc.tile_pool(name="p2", bufs=3) as p2, \
             tc.tile_pool(name="p2ps", bufs=1, space="PSUM") as p2ps, \
             tc.tile_pool(name="c2pool", bufs=1) as c2pool:
            ctx1e = c2pool.tile([128, B, 3, NMT, 66], bf16)
            for r in range(NCORES):
                for b in range(B):
                    for i in range(3):
                        nc.sync.dma_start(
                            ctx1e[:, b, i, r * (OWN // 128):(r + 1) * (OWN // 128), 0:HD],
                            ctx1_gath[r, b, i].rearrange("(q p) d -> p q d", p=128))
            nc.vector.memset(ctx1e[:, :, :, :, HD:HD + 2], 1.0)
            ones1 = c2pool.tile([1, 64], fp32)
            nc.vector.memset(ones1[:], 1.0)

            for p in POSE_PAIRS:
                (bA, hA), (bB_, hB) = PAIRS[p]
                iA, iB = hA - 3, hB - 3
                ps_c2a = p2ps.tile([65, OWN], fp32, tag="c2psA", bufs=1, space="PSUM")
                ps_c2b = p2ps.tile([65, OWN], fp32, tag="c2psB", bufs=1, space="PSUM")
                for mt in range(NMT):
                    ps = p2ps.tile([128, 2 * OWN], fp32, tag="pps", bufs=2, space="PSUM")
                    nc.tensor.matmul(
                        ps[:, 0:OWN], lhsT=kT[p][0:64, mt * 128:(mt + 1) * 128],
                        rhs=qT[p][0:64], tile_position=(0, 0))
                    nc.tensor.matmul(
                        ps[:, OWN:], lhsT=kT[p][64:128, mt * 128:(mt + 1) * 128],
                        rhs=qT[p][64:128], tile_position=(64, 0))
                    et2 = p2.tile([128, 2 * OWN], bf16, tag="et2")
                    nc.scalar.activation(out=et2[:], in_=ps[:], func=AF.Exp)
                    nc.tensor.matmul(
                        ps_c2a[:], lhsT=ctx1e[:, bA, iA, mt, 0:65],
                        rhs=et2[:, 0:OWN], start=(mt == 0), stop=(mt == NMT - 1),
                        skip_group_check=True)
                    nc.tensor.matmul(
                        ps_c2b[:], lhsT=ctx1e[:, bB_, iB, mt, 0:65],
                        rhs=et2[:, OWN:], start=(mt == 0), stop=(mt == NMT - 1),
                        skip_group_check=True)
                for half, psx, (b_, i_) in ((0, ps_c2a, (bA, iA)), (1, ps_c2b, (bB_, iB))):
                    den = p2.tile([1, OWN], fp32, tag="den")
                    nc.vector.tensor_copy(out=den[:], in_=psx[64:65, :])
                    nc.vector.reciprocal(out=den[:], in_=den[:])
                    psb = p2ps.tile([64, OWN], fp32, tag="rbps", bufs=1, space="PSUM")
                    nc.tensor.matmul(psb[:], lhsT=ones1[:], rhs=den[:])
                    c2u = p2.tile([64, OWN], fp32, tag="c2u")
                    nc.vector.tensor_copy(out=c2u[:], in_=psx[0:64, :])
                    # c2nT tensor index: pose pair order (p-3); rows by member half
                    nc.vector.tensor_tensor(
                        c2nT[p - 3][64 * half:64 * half + 64, :],
                        c2u[:], psb[:], ALU.mult)

        # ---------------- P3: proj + LN2 + MLP ----------------
        with tc.tile_pool(name="p3", bufs=3) as p3, \
             tc.tile_pool(name="p3s", bufs=1) as p3s, \
             tc.tile_pool(name="p3ps", bufs=1, space="PSUM") as p3ps:
            pw01_sb = p3s.tile([128, C], bf16)
            nc.sync.dma_start(pw01_sb[:], pwT01[:])
            pw2_sb = p3s.tile([128, C], bf16)
            nc.sync.dma_start(pw2_sb[:], pwT2[:])
            pbB_sb = p3s.tile([128, C], fp32)
            nc.sync.dma_start(pbB_sb[:], pbB[:])
            fc1w_sb = p3s.tile([128, 3, MLP_H], bf16)
            nc.sync.dma_start(fc1w_sb[:], fc1wT.rearrange("(o p) q -> p o q", p=128))
            fc1b_sb = p3s.tile([128, MLP_H // 128], fp32)
            nc.sync.dma_start(fc1b_sb[:], fc1b[:])
            fc2w_sb = p3s.tile([128, MLP_H // 128, C], bf16)
            nc.sync.dma_start(fc2w_sb[:], fc2wT.rearrange("(o p) c -> p o c", p=128))
            fc2b_sb = p3s.tile([128, C], fp32)
            nc.sync.dma_start(fc2b_sb[:], fc2bB[:])

            x2_sb = p3s.tile([128, B * OWN // 128, C], fp32)
            h2T = p3s.tile([128, 3, B * OWN], bf16)
            c2_of_b = {0: c2nT[0], 1: c2nT[1]}
            for b in range(B):
                for qc in range(OWN // 128):
                    ps = p3ps.tile([128, C], fp32, tag="mmps", bufs=3, space="PSUM")
                    nc.tensor.matmul(
                        ps[:], lhsT=c2_of_b[b][:, qc * 128:(qc + 1) * 128],
                        rhs=pw01_sb[:], start=True, stop=False,
                        skip_group_check=True)
                    base = 64 * b
                    nc.tensor.matmul(
                        ps[:], lhsT=c2nT[2][base:base + 64, qc * 128:(qc + 1) * 128],
                        rhs=pw2_sb[base:base + 64], start=False, stop=True,
                        skip_group_check=True)
                    ti = b * (OWN // 128) + qc
                    xt = x2_sb[:, ti]
                    nc.vector.tensor_tensor(xt, ps[:], x_own_sb[:, ti], ALU.add)
                    nc.vector.tensor_tensor(xt, xt, pbB_sb[:], ALU.add)
                    st = p3.tile([128, 6], fp32, tag="st2")
                    mv = p3.tile([128, 2], fp32, tag="mv2")
                    nc.vector.bn_stats(out=st[:], in_=xt)
                    nc.vector.bn_aggr(out=mv[:], in_=st[:])
                    rstd = p3.tile([128, 1], fp32, tag="rstd2")
                    nc.scalar.activation(out=rstd[:], in_=mv[:, 1:2], func=AF.Sqrt,
                                         bias=eps_sb[:], scale=1.0)
                    nc.vector.reciprocal(out=rstd[:], in_=rstd[:])
                    h2 = p3.tile([128, C], bf16, tag="h2")
                    nc.vector.tensor_scalar(
                        h2[:], xt, mv[:, 0:1], rstd[:], ALU.subtract, ALU.mult)
                    for cc in range(3):
                        pt = p3ps.tile([128, 128], bf16, tag="h2tr", bufs=3,
                                       space="PSUM")
                        nc.tensor.transpose(pt[:], h2[:, cc * 128:(cc + 1) * 128], ident)
                        nc.vector.tensor_copy(
                            out=h2T[:, cc, ti * 128:(ti + 1) * 128], in_=pt[:])

            g_sb = p3s.tile([128, MLP_H // 128, B * OWN], bf16)
            for s in range(MLP_H // 128):
                for qh in range(B * OWN // 512):
                    ps = p3ps.tile([128, 512], fp32, tag="mmps", bufs=3, space="PSUM")
                    for cc in range(3):
                        nc.tensor.matmul(
                            ps[:], lhsT=fc1w_sb[:, cc, s * 128:(s + 1) * 128],
                            rhs=h2T[:, cc, qh * 512:(qh + 1) * 512],
                            start=(cc == 0), stop=(cc == 2))
                    nc.scalar.activation(
                        out=g_sb[:, s, qh * 512:(qh + 1) * 512], in_=ps[:],
                        func=AF.Gelu, bias=fc1b_sb[:, s:s + 1], scale=1.0)

            for ti in range(B * OWN // 128):
                ps = p3ps.tile([128, C], fp32, tag="mmps", bufs=3, space="PSUM")
                for s in range(MLP_H // 128):
                    nc.tensor.matmul(
                        ps[:], lhsT=g_sb[:, s, ti * 128:(ti + 1) * 128],
                        rhs=fc2w_sb[:, s], start=(s == 0),
                        stop=(s == MLP_H // 128 - 1))
                yt = p3.tile([128, C], fp32, tag="yt")
                nc.vector.tensor_tensor(yt[:], ps[:], x2_sb[:, ti], ALU.add)
                nc.vector.tensor_tensor(yt[:], yt[:], fc2b_sb[:], ALU.add)
                nc.sync.dma_start(
                    y_out.rearrange("(o p) c -> p o c", p=128)[:, ti], yt[:])

    nc.finalize()
    return nc


def prepare_inputs(inputs):
    """Host-side: fold gammas/betas/score-scale into weights, build per-core maps."""
    import ml_dtypes
    bf16 = ml_dtypes.bfloat16

    x = np.asarray(inputs["x"], np.float32)
    qkv_w = np.asarray(inputs["qkv_w"], np.float32)
    proj_w = np.asarray(inputs["proj_w"], np.float32)
    proj_b = np.asarray(inputs["proj_b"], np.float32)
    g1 = np.asarray(inputs["gamma1"], np.float32)
    b1 = np.asarray(inputs["beta1"], np.float32)
    g2 = np.asarray(inputs["gamma2"], np.float32)
    b2 = np.asarray(inputs["beta2"], np.float32)
    fc1_w = np.asarray(inputs["fc1_w"], np.float32)
    fc1_b = np.asarray(inputs["fc1_b"], np.float32)
    fc2_w = np.asarray(inputs["fc2_w"], np.float32)
    fc2_b = np.asarray(inputs["fc2_b"], np.float32)

    wp = qkv_w * g1[None, :]
    qkv_bias = qkv_w @ b1
    wp[:C] *= SCALE
    qkv_bias[:C] *= SCALE
    wT = np.ascontiguousarray(wp.T)

    def pair_bias(sec):
        out = np.zeros((128, 6), np.float32)
        for p, ((bA, hA), (bB_, hB)) in enumerate(PAIRS):
            out[0:64, p] = qkv_bias[sec + hA * HD: sec + (hA + 1) * HD]
            out[64:128, p] = qkv_bias[sec + hB * HD: sec + (hB + 1) * HD]
        return out

    qb_q = pair_bias(0)
    qb_k = pair_bias(C)
    vb = np.broadcast_to(qkv_bias[2 * C + HALF:], (128, HALF)).copy()

    pwT = np.ascontiguousarray(proj_w.T)
    pwT01 = pwT[0:128]
    pwT2 = np.concatenate([pwT[128:192], pwT[128:192]], axis=0)
    pbB = np.broadcast_to(proj_b, (128, C)).copy()

    fc1wp = fc1_w * g2[None, :]
    fc1bp = fc1_b + fc1_w @ b2
    fc1wT = np.ascontiguousarray(fc1wp.T)
    fc1b_t = np.ascontiguousarray(fc1bp.reshape(MLP_H // 128, 128).T)
    fc2wT = np.ascontiguousarray(fc2_w.T)
    fc2bB = np.broadcast_to(fc2_b, (128, C)).copy()

    shared = {
        "x_full": np.ascontiguousarray(x.reshape(B * N, C)),
        "wT": wT.astype(bf16),
        "qb_q": qb_q, "qb_k": qb_k, "vb": vb,
        "pwT01": pwT01.astype(bf16), "pwT2": pwT2.astype(bf16),
        "pbB": pbB.astype(np.float32),
        "fc1wT": fc1wT.astype(bf16), "fc1b": fc1b_t.astype(np.float32),
        "fc2wT": fc2wT.astype(bf16), "fc2bB": fc2bB.astype(np.float32),
    }
    in_maps = []
    for c in range(NCORES):
        m = dict(shared)
        m["x_own"] = np.ascontiguousarray(
            x[:, c * OWN:(c + 1) * OWN, :].reshape(B * OWN, C))
        in_maps.append(m)
    return in_maps


_CACHE = {}


def kernel(**inputs):
    from concourse.bass_utils import run_bass_kernel_spmd

    if "nc" not in _CACHE:
        _CACHE["nc"] = build_kernel()
    nc = _CACHE["nc"]
    in_maps = prepare_inputs(inputs)
    res = run_bass_kernel_spmd(nc, in_maps, core_ids=list(range(NCORES)))
    _CACHE["last_results"] = res

    attn_obj = np.empty((B, 3, N, N), np.float32)
    y = np.empty((B, N, C), np.float32)
    for c in range(NCORES):
        r = res.results[c]
        attn_obj[:, :, c * OWN:(c + 1) * OWN, :] = r["attn_out"]
        y[:, c * OWN:(c + 1) * OWN, :] = r["y_out"].reshape(B, N // NCORES, C)
    return (y, attn_obj)


# revision 14
# speedup vs baseline: 93.4707x; 93.4707x over previous
"""Trainium2 Bass kernel for nn_Block_15006615734251 (dense transformer block
with chained sigmoid/softmax attention), SPMD over 8 NeuronCores.

Sharding: query rows split 8 ways (512 rows/core/batch). Each core computes
the full K/V projections (replicated), its own rows of attn_obj (sigmoid
scores written straight to HBM), its own rows of the obj->pose context chain,
and the epilogue (proj/LN2/MLP) for its own rows. The only cross-core
exchange is an AllGather of ctx1 = sigmoid(S_obj) @ V  ([2,3,4096,64] bf16).
"""

import numpy as np

B, N, C, H = 2, 4096, 384, 6
HD = C // H            # 64
HALF = C // 2          # 192
MLP_H = 4 * C          # 1536
EPS = 1e-5
NCORES = 8
OWN = N // NCORES      # 512 query rows per core per batch
NMT = N // 128         # 32 key tiles per batch
SCALE = HD ** -0.5

# Head pairs: each pair occupies one 128-partition tensor (member0 in
# partitions 0:64, member1 in 64:128). Obj heads (0-2) pair with obj heads,
# pose (3-5) with pose, so the sigmoid and exp phases stay homogeneous.
PAIRS = [
    ((0, 0), (0, 1)),  # obj
    ((1, 0), (1, 1)),  # obj
    ((0, 2), (1, 2)),  # obj (cross-batch)
    ((0, 3), (0, 4)),  # pose
    ((1, 3), (1, 4)),  # pose
    ((0, 5), (1, 5)),  # pose
]
OBJ_PAIRS = [0, 1, 2]
POSE_PAIRS = [3, 4, 5]


def build_kernel():
    import contextlib
    import concourse.bass as bass
    import concourse.tile as tile
    from concourse import bacc, mybir
    from concourse.masks import make_identity
    from concourse.tile import add_dep_helper

    fp32 = mybir.dt.float32
    bf16 = mybir.dt.bfloat16
    AF = mybir.ActivationFunctionType
    ALU = mybir.AluOpType

    nc = bacc.Bacc("TRN2", target_bir_lowering=False, debug=False,
                   enable_asserts=True, num_devices=NCORES)

    # ---- I/O ----
    x_full = nc.dram_tensor("x_full", [B * N, C], fp32, kind="ExternalInput")
    x_own = nc.dram_tensor("x_own", [B * OWN, C], fp32, kind="ExternalInput")
    wT = nc.dram_tensor("wT", [C, 3 * C], bf16, kind="ExternalInput")
    qb_q = nc.dram_tensor("qb_q", [128, 6], fp32, kind="ExternalInput")
    qb_k = nc.dram_tensor("qb_k", [128, 6], fp32, kind="ExternalInput")
    vb = nc.dram_tensor("vb", [128, HALF], fp32, kind="ExternalInput")
    pwT01 = nc.dram_tensor("pwT01", [128, C], bf16, kind="ExternalInput")
    pwT2 = nc.dram_tensor("pwT2", [128, C], bf16, kind="ExternalInput")
    pbB = nc.dram_tensor("pbB", [128, C], fp32, kind="ExternalInput")
    fc1wT = nc.dram_tensor("fc1wT", [C, MLP_H], bf16, kind="ExternalInput")
    fc1b = nc.dram_tensor("fc1b", [128, MLP_H // 128], fp32, kind="ExternalInput")
    fc2wT = nc.dram_tensor("fc2wT", [MLP_H, C], bf16, kind="ExternalInput")
    fc2bB = nc.dram_tensor("fc2bB", [128, C], fp32, kind="ExternalInput")

    attn_out = nc.dram_tensor("attn_out", [B, 3, OWN, N], fp32, kind="ExternalOutput")
    y_out = nc.dram_tensor("y_out", [B * OWN, C], fp32, kind="ExternalOutput")

    with tile.TileContext(nc) as tc, contextlib.ExitStack() as es:
        # ---------------- persistent pools ----------------
        singles = es.enter_context(tc.tile_pool(name="singles", bufs=1))
        persist = es.enter_context(tc.tile_pool(name="persist", bufs=1))
        dram = es.enter_context(tc.tile_pool(name="dram", bufs=1, space="DRAM"))
        psp = es.enter_context(tc.tile_pool(name="psp", bufs=1, space="PSUM"))

        ident = singles.tile([128, 128], bf16)
        make_identity(nc, ident)
        identf = singles.tile([128, 128], fp32)
        make_identity(nc, identf)

        wT_sb = singles.tile([128, 3, 3 * C], bf16)
        nc.sync.dma_start(wT_sb[:], wT.rearrange("(o p) q -> p o q", p=128))
        qbq_sb = singles.tile([128, 6], fp32)
        nc.sync.dma_start(qbq_sb[:], qb_q[:])
        qbk_sb = singles.tile([128, 6], fp32)
        nc.sync.dma_start(qbk_sb[:], qb_k[:])
        vb_sb = singles.tile([128, HALF], fp32)
        nc.sync.dma_start(vb_sb[:], vb[:])
        eps_sb = singles.tile([128, 1], fp32)
        nc.vector.memset(eps_sb[:], EPS)

        x_own_sb = persist.tile([128, B * OWN // 128, C], fp32)
        nc.sync.dma_start(x_own_sb[:], x_own.rearrange("(o p) c -> p o c", p=128))

        kT = [persist.tile([128, N], bf16, name=f"kT{p}") for p in range(6)]
        qT = [persist.tile([128, OWN], bf16, name=f"qT{p}") for p in range(6)]
        v_sb = [persist.tile([128, NMT, HALF], bf16, name=f"v{b}") for b in range(B)]
        c2nT = [persist.tile([128, OWN], bf16, name=f"c2nT{i}") for i in range(3)]

        # ---------------- P0: LN1 -> h^T, then QKV ----------------
        NT_ALL = B * N // 128
        NT_OWN = B * OWN // 128

        with tc.tile_pool(name="hT_pool", bufs=1) as hT_pool, \
             tc.tile_pool(name="p0", bufs=2) as p0:
            hT = hT_pool.tile([128, 3, B * N], bf16)
            hoT = hT_pool.tile([128, 3, B * OWN], bf16)

            def ln_tiles(src_ap, ntiles, dst, group=4):
                for g in range(0, ntiles, group):
                    cnt = min(group, ntiles - g)
                    xg = p0.tile([128, group, C], fp32, tag="xg", bufs=2)
                    nc.sync.dma_start(xg[:, :cnt], src_ap[:, g:g + cnt])
                    st = p0.tile([128, group, 6], fp32, tag="st", bufs=2)
                    mv = p0.tile([128, group, 2], fp32, tag="mv", bufs=2)
                    for j in range(cnt):
                        nc.vector.bn_stats(out=st[:, j], in_=xg[:, j])
                        nc.vector.bn_aggr(out=mv[:, j], in_=st[:, j])
                    rstd = p0.tile([128, group], fp32, tag="rstd", bufs=2)
                    nc.scalar.activation(out=rstd[:, :cnt], in_=mv[:, :cnt, 1],
                                         func=AF.Sqrt, bias=eps_sb[:], scale=1.0)
                    nc.vector.reciprocal(out=rstd[:, :cnt], in_=rstd[:, :cnt])
                    for j in range(cnt):
                        hj = p0.tile([128, C], fp32, tag="hj", bufs=2)
                        nc.vector.tensor_scalar(
                            hj[:], xg[:, j], mv[:, j, 0:1], rstd[:, j:j + 1],
                            ALU.subtract, ALU.mult)
                        for cc in range(3):
                            pt = psp.tile([128, 128], fp32, tag="small", bufs=2,
                                          space="PSUM")
                            nc.tensor.transpose(pt[:], hj[:, cc * 128:(cc + 1) * 128],
                                                identf)
                            nc.vector.tensor_copy(
                                out=dst[:, cc, (g + j) * 128:(g + j + 1) * 128],
                                in_=pt[:])

            ln_tiles(x_full.rearrange("(o p) c -> p o c", p=128), NT_ALL, hT)
            ln_tiles(x_own.rearrange("(o p) c -> p o c", p=128), NT_OWN, hoT)

            def k_col(h):
                return C + h * HD

            def q_col(h):
                return h * HD

            for p, ((bA, hA), (bB_, hB)) in enumerate(PAIRS):
                same_b = (bA == bB_)
                for j in range(N // 512):
                    ps = psp.tile([128, 512], fp32, tag="big", bufs=2, space="PSUM")
                    for cc in range(3):
                        if same_b:
                            nc.tensor.matmul(
                                ps[:], lhsT=wT_sb[:, cc, k_col(hA):k_col(hA) + 128],
                                rhs=hT[:, cc, bA * N + j * 512: bA * N + (j + 1) * 512],
                                start=(cc == 0), stop=(cc == 2))
                        else:
                            nc.tensor.matmul(
                                ps[0:64], lhsT=wT_sb[:, cc, k_col(hA):k_col(hA) + 64],
                                rhs=hT[:, cc, bA * N + j * 512: bA * N + (j + 1) * 512],
                                start=(cc == 0), stop=(cc == 2), tile_position=(0, 0))
                            nc.tensor.matmul(
                                ps[64:128], lhsT=wT_sb[:, cc, k_col(hB):k_col(hB) + 64],
                                rhs=hT[:, cc, bB_ * N + j * 512: bB_ * N + (j + 1) * 512],
                                start=(cc == 0), stop=(cc == 2), tile_position=(0, 64))
                    nc.vector.tensor_scalar(
                        kT[p][:, j * 512:(j + 1) * 512], ps[:],
                        qbk_sb[:, p:p + 1], None, ALU.add)
                ps = psp.tile([128, 512], fp32, tag="big", bufs=2, space="PSUM")
                for cc in range(3):
                    if same_b:
                        nc.tensor.matmul(
                            ps[:], lhsT=wT_sb[:, cc, q_col(hA):q_col(hA) + 128],
                            rhs=hoT[:, cc, bA * OWN:(bA + 1) * OWN],
                            start=(cc == 0), stop=(cc == 2))
                    else:
                        nc.tensor.matmul(
                            ps[0:64], lhsT=wT_sb[:, cc, q_col(hA):q_col(hA) + 64],
                            rhs=hoT[:, cc, bA * OWN:(bA + 1) * OWN],
                            start=(cc == 0), stop=(cc == 2), tile_position=(0, 0))
                        nc.tensor.matmul(
                            ps[64:128], lhsT=wT_sb[:, cc, q_col(hB):q_col(hB) + 64],
                            rhs=hoT[:, cc, bB_ * OWN:(bB_ + 1) * OWN],
                            start=(cc == 0), stop=(cc == 2), tile_position=(0, 64))
                nc.vector.tensor_scalar(
                    qT[p][:], ps[:], qbq_sb[:, p:p + 1], None, ALU.add)

            for b in range(B):
                for mt in range(NMT):
                    ps = psp.tile([128, HALF], fp32, tag="small", bufs=2, space="PSUM")
                    for cc in range(3):
                        nc.tensor.matmul(
                            ps[:],
                            lhsT=hT[:, cc, b * N + mt * 128: b * N + (mt + 1) * 128],
                            rhs=wT_sb[:, cc, 2 * C + HALF: 3 * C],
                            start=(cc == 0), stop=(cc == 2))
                    nc.vector.tensor_tensor(v_sb[b][:, mt], ps[:], vb_sb[:], ALU.add)

        # ---------------- P1a: obj chains (sigmoid -> ctx1 -> gather) ----------
        gath = [dram.tile([NCORES, 2, OWN, HD], bf16, addr_space="Shared",
                          name=f"gath{i}") for i in range(3)]
        loc = [dram.tile([2, OWN, HD], bf16, name=f"loc{i}") for i in range(3)]

        with tc.tile_pool(name="p1", bufs=3) as p1, \
             tc.tile_pool(name="p2", bufs=3) as p2, \
             tc.tile_pool(name="c1pool", bufs=1) as c1pool:
            ctx1e = [c1pool.tile([128, 2, NMT, 66], bf16, name=f"ctx1e{i}")
                     for i in range(3)]
            for p in OBJ_PAIRS:
                (bA, hA), (bB_, hB) = PAIRS[p]
                iA, iB = hA, hB
                ps_c1 = psp.tile([128, 512], fp32, tag="accum", bufs=2, space="PSUM")
                for mt in range(NMT):
                    ps = psp.tile([128, 2 * OWN], fp32, tag="big", bufs=2, space="PSUM")
                    nc.tensor.matmul(
                        ps[:, 0:OWN], lhsT=kT[p][0:64, mt * 128:(mt + 1) * 128],
                        rhs=qT[p][0:64], tile_position=(0, 0))
                    nc.tensor.matmul(
                        ps[:, OWN:], lhsT=kT[p][64:128, mt * 128:(mt + 1) * 128],
                        rhs=qT[p][64:128], tile_position=(64, 0))
                    at2 = p1.tile([128, 2 * OWN], bf16, tag="at2")
                    nc.scalar.activation(out=at2[:], in_=ps[:], func=AF.Sigmoid)
                    nc.tensor.matmul(
                        ps_c1[0:64], lhsT=v_sb[bA][:, mt, iA * HD:(iA + 1) * HD],
                        rhs=at2[:, 0:OWN], start=(mt == 0), stop=(mt == NMT - 1),
                        tile_position=(0, 0), skip_group_check=True)
                    nc.tensor.matmul(
                        ps_c1[64:128], lhsT=v_sb[bB_][:, mt, iB * HD:(iB + 1) * HD],
                        rhs=at2[:, OWN:], start=(mt == 0), stop=(mt == NMT - 1),
                        tile_position=(0, 64), skip_group_check=True)
                c1sb = p1.tile([128, 512], bf16, tag="c1sb")
                nc.vector.tensor_copy(out=c1sb[:], in_=ps_c1[:])
                c1row = c1pool.tile([128, 2, OWN // 128, HD], bf16, tag=f"c1row{p}")
                for qc in range(OWN // 128):
                    pt = psp.tile([128, 128], bf16, tag="small", bufs=2, space="PSUM")
                    nc.tensor.transpose(pt[:], c1sb[:, qc * 128:(qc + 1) * 128], ident)
                    nc.vector.tensor_copy(out=c1row[:, 0, qc], in_=pt[:, 0:64])
                    nc.vector.tensor_copy(out=c1row[:, 1, qc], in_=pt[:, 64:128])
                nc.sync.dma_start(
                    loc[p].rearrange("m (q pp) d -> pp m q d", pp=128), c1row[:])
                nc.gpsimd.collective_compute(
                    "AllGather", ALU.bypass,
                    replica_groups=[list(range(NCORES))],
                    ins=[loc[p].opt()], outs=[gath[p].opt()])
                for r in range(NCORES):
                    for m in range(2):
                        nc.sync.dma_start(
                            ctx1e[p][:, m, r * (OWN // 128):(r + 1) * (OWN // 128), 0:HD],
                            gath[p][r, m].rearrange("(q pp) d -> pp q d", pp=128))
                nc.vector.memset(ctx1e[p][:, :, :, HD:HD + 2], 1.0)

            # ---------------- P2: pose (exp softmax) + ctx2 ----------------
            ones1 = c1pool.tile([1, 64], fp32)
            nc.vector.memset(ones1[:], 1.0)
            for p in POSE_PAIRS:
                (bA, hA), (bB_, hB) = PAIRS[p]
                op = p - 3  # matching obj pair index / ctx1e index
                ps_c2a = psp.tile([65, OWN], fp32, tag="accum", bufs=2, space="PSUM")
                ps_c2b = psp.tile([65, OWN], fp32, tag="accum", bufs=2, space="PSUM")
                for mt in range(NMT):
                    ps = psp.tile([128, 2 * OWN], fp32, tag="big", bufs=2, space="PSUM")
                    nc.tensor.matmul(
                        ps[:, 0:OWN], lhsT=kT[p][0:64, mt * 128:(mt + 1) * 128],
                        rhs=qT[p][0:64], tile_position=(0, 0))
                    nc.tensor.matmul(
                        ps[:, OWN:], lhsT=kT[p][64:128, mt * 128:(mt + 1) * 128],
                        rhs=qT[p][64:128], tile_position=(64, 0))
                    et2 = p2.tile([128, 2 * OWN], bf16, tag="et2", bufs=4)
                    nc.scalar.activation(out=et2[:], in_=ps[:], func=AF.Exp)
                    nc.tensor.matmul(
                        ps_c2a[:], lhsT=ctx1e[op][:, 0, mt, 0:65],
                        rhs=et2[:, 0:OWN], start=(mt == 0), stop=(mt == NMT - 1),
                        skip_group_check=True)
                    nc.tensor.matmul(
                        ps_c2b[:], lhsT=ctx1e[op][:, 1, mt, 0:65],
                        rhs=et2[:, OWN:], start=(mt == 0), stop=(mt == NMT - 1),
                        skip_group_check=True)
                for half, psx in ((0, ps_c2a), (1, ps_c2b)):
                    den = p2.tile([1, OWN], fp32, tag="den")
                    nc.vector.tensor_copy(out=den[:], in_=psx[64:65, :])
                    nc.vector.reciprocal(out=den[:], in_=den[:])
                    psb = psp.tile([64, OWN], fp32, tag="small", bufs=2, space="PSUM")
                    nc.tensor.matmul(psb[:], lhsT=ones1[:], rhs=den[:])
                    c2u = p2.tile([64, OWN], fp32, tag="c2u")
                    nc.vector.tensor_copy(out=c2u[:], in_=psx[0:64, :])
                    nc.vector.tensor_tensor(
                        c2nT[op][64 * half:64 * half + 64, :],
                        c2u[:], psb[:], ALU.mult)

            # ---------------- P1b: obj row-major scores -> attn_obj ----------
            for p in OBJ_PAIRS:
                (bA, hA), (bB_, hB) = PAIRS[p]
                iA, iB = hA, hB
                for half in range(2):
                    base = 64 * half
                    b_, i_ = (bA, iA) if half == 0 else (bB_, iB)
                    for nt in range(OWN // 128):
                        for mh in range(N // 1024):
                            ps = psp.tile([128, 2 * OWN], fp32, tag="big", bufs=2,
                                          space="PSUM")
                            for mc in range(2):
                                nc.tensor.matmul(
                                    ps[:, mc * 512:(mc + 1) * 512],
                                    lhsT=qT[p][base:base + 64, nt * 128:(nt + 1) * 128],
                                    rhs=kT[p][base:base + 64,
                                              mh * 1024 + mc * 512:
                                              mh * 1024 + (mc + 1) * 512],
                                    tile_position=(base, 0))
                            stg = p1.tile([128, 1024], fp32, tag="stg", bufs=4)
                            nc.scalar.activation(out=stg[:], in_=ps[:], func=AF.Sigmoid)
                            nc.sync.dma_start(
                                attn_out[b_, i_, nt * 128:(nt + 1) * 128,
                                         mh * 1024:(mh + 1) * 1024],
                                stg[:])

        # ---------------- P3: proj + LN2 + MLP ----------------
        with tc.tile_pool(name="p3", bufs=3) as p3, \
             tc.tile_pool(name="p3s", bufs=1) as p3s:
            pw01_sb = p3s.tile([128, C], bf16)
            nc.sync.dma_start(pw01_sb[:], pwT01[:])
            pw2_sb = p3s.tile([128, C], bf16)
            nc.sync.dma_start(pw2_sb[:], pwT2[:])
            pbB_sb = p3s.tile([128, C], fp32)
            nc.sync.dma_start(pbB_sb[:], pbB[:])
            fc1w_sb = p3s.tile([128, 3, MLP_H], bf16)
            nc.sync.dma_start(fc1w_sb[:], fc1wT.rearrange("(o p) q -> p o q", p=128))
            fc1b_sb = p3s.tile([128, MLP_H // 128], fp32)
            nc.sync.dma_start(fc1b_sb[:], fc1b[:])
            fc2w_sb = p3s.tile([128, MLP_H // 128, C], bf16)
            nc.sync.dma_start(fc2w_sb[:], fc2wT.rearrange("(o p) c -> p o c", p=128))
            fc2b_sb = p3s.tile([128, C], fp32)
            nc.sync.dma_start(fc2b_sb[:], fc2bB[:])

            x2_sb = p3s.tile([128, B * OWN // 128, C], fp32)
            h2T = p3s.tile([128, 3, B * OWN], bf16)
            c2_of_b = {0: c2nT[0], 1: c2nT[1]}
            for b in range(B):
                for qc in range(OWN // 128):
                    ps = psp.tile([128, C], fp32, tag="big", bufs=2, space="PSUM")
                    nc.tensor.matmul(
                        ps[:], lhsT=c2_of_b[b][:, qc * 128:(qc + 1) * 128],
                        rhs=pw01_sb[:], start=True, stop=False,
                        skip_group_check=True)
                    base = 64 * b
                    nc.tensor.matmul(
                        ps[:], lhsT=c2nT[2][base:base + 64, qc * 128:(qc + 1) * 128],
                        rhs=pw2_sb[base:base + 64], start=False, stop=True,
                        skip_group_check=True)
                    ti = b * (OWN // 128) + qc
                    xt = x2_sb[:, ti]
                    nc.vector.tensor_tensor(xt, ps[:], x_own_sb[:, ti], ALU.add)
                    nc.vector.tensor_tensor(xt, xt, pbB_sb[:], ALU.add)
                    st = p3.tile([128, 6], fp32, tag="st2")
                    mv = p3.tile([128, 2], fp32, tag="mv2")
                    nc.vector.bn_stats(out=st[:], in_=xt)
                    nc.vector.bn_aggr(out=mv[:], in_=st[:])
                    rstd = p3.tile([128, 1], fp32, tag="rstd2")
                    nc.scalar.activation(out=rstd[:], in_=mv[:, 1:2], func=AF.Sqrt,
                                         bias=eps_sb[:], scale=1.0)
                    nc.vector.reciprocal(out=rstd[:], in_=rstd[:])
                    h2 = p3.tile([128, C], bf16, tag="h2")
                    nc.vector.tensor_scalar(
                        h2[:], xt, mv[:, 0:1], rstd[:], ALU.subtract, ALU.mult)
                    for cc in range(3):
                        pt = psp.tile([128, 128], bf16, tag="small", bufs=2,
                                      space="PSUM")
                        nc.tensor.transpose(pt[:], h2[:, cc * 128:(cc + 1) * 128], ident)
                        nc.vector.tensor_copy(
                            out=h2T[:, cc, ti * 128:(ti + 1) * 128], in_=pt[:])

            g_sb = p3s.tile([128, MLP_H // 128, B * OWN], bf16)
            for s in range(MLP_H // 128):
                for qh in range(B * OWN // 512):
                    ps = psp.tile([128, 512], fp32, tag="big", bufs=2, space="PSUM")
                    for cc in range(3):
                        nc.tensor.matmul(
                            ps[:], lhsT=fc1w_sb[:, cc, s * 128:(s + 1) * 128],
                            rhs=h2T[:, cc, qh * 512:(qh + 1) * 512],
                            start=(cc == 0), stop=(cc == 2))
                    nc.scalar.activation(
                        out=g_sb[:, s, qh * 512:(qh + 1) * 512], in_=ps[:],
                        func=AF.Gelu, bias=fc1b_sb[:, s:s + 1], scale=1.0)

            for ti in range(B * OWN // 128):
                ps = psp.tile([128, C], fp32, tag="big", bufs=2, space="PSUM")
                for s in range(MLP_H // 128):
                    nc.tensor.matmul(
                        ps[:], lhsT=g_sb[:, s, ti * 128:(ti + 1) * 128],
                        rhs=fc2w_sb[:, s], start=(s == 0),
                        stop=(s == MLP_H // 128 - 1))
                yt = p3.tile([128, C], fp32, tag="yt")
                nc.vector.tensor_tensor(yt[:], ps[:], x2_sb[:, ti], ALU.add)
                nc.vector.tensor_tensor(yt[:], yt[:], fc2b_sb[:], ALU.add)
                nc.sync.dma_start(
                    y_out.rearrange("(o p) c -> p o c", p=128)[:, ti], yt[:])

    nc.finalize()
    return nc


def prepare_inputs(inputs):
    """Host-side: fold gammas/betas/score-scale into weights, build per-core maps."""
    import ml_dtypes
    bf16 = ml_dtypes.bfloat16

    x = np.asarray(inputs["x"], np.float32)
    qkv_w = np.asarray(inputs["qkv_w"], np.float32)
    proj_w = np.asarray(inputs["proj_w"], np.float32)
    proj_b = np.asarray(inputs["proj_b"], np.float32)
    g1 = np.asarray(inputs["gamma1"], np.float32)
    b1 = np.asarray(inputs["beta1"], np.float32)
    g2 = np.asarray(inputs["gamma2"], np.float32)
    b2 = np.asarray(inputs["beta2"], np.float32)
    fc1_w = np.asarray(inputs["fc1_w"], np.float32)
    fc1_b = np.asarray(inputs["fc1_b"], np.float32)
    fc2_w = np.asarray(inputs["fc2_w"], np.float32)
    fc2_b = np.asarray(inputs["fc2_b"], np.float32)

    wp = qkv_w * g1[None, :]
    qkv_bias = qkv_w @ b1
    wp[:C] *= SCALE
    qkv_bias[:C] *= SCALE
    wT = np.ascontiguousarray(wp.T)

    def pair_bias(sec):
        out = np.zeros((128, 6), np.float32)
        for p, ((bA, hA), (bB_, hB)) in enumerate(PAIRS):
            out[0:64, p] = qkv_bias[sec + hA * HD: sec + (hA + 1) * HD]
            out[64:128, p] = qkv_bias[sec + hB * HD: sec + (hB + 1) * HD]
        return out

    qb_q = pair_bias(0)
    qb_k = pair_bias(C)
    vb = np.broadcast_to(qkv_bias[2 * C + HALF:], (128, HALF)).copy()

    pwT = np.ascontiguousarray(proj_w.T)
    pwT01 = pwT[0:128]
    pwT2 = np.concatenate([pwT[128:192], pwT[128:192]], axis=0)
    pbB = np.broadcast_to(proj_b, (128, C)).copy()

    fc1wp = fc1_w * g2[None, :]
    fc1bp = fc1_b + fc1_w @ b2
    fc1wT = np.ascontiguousarray(fc1wp.T)
    fc1b_t = np.ascontiguousarray(fc1bp.reshape(MLP_H // 128, 128).T)
    fc2wT = np.ascontiguousarray(fc2_w.T)
    fc2bB = np.broadcast_to(fc2_b, (128, C)).copy()

    shared = {
        "x_full": np.ascontiguousarray(x.reshape(B * N, C)),
        "wT": wT.astype(bf16),
        "qb_q": qb_q, "qb_k": qb_k, "vb": vb,
        "pwT01": pwT01.astype(bf16), "pwT2": pwT2.astype(bf16),
        "pbB": pbB.astype(np.float32),
        "fc1wT": fc1wT.astype(bf16), "fc1b": fc1b_t.astype(np.float32),
        "fc2wT": fc2wT.astype(bf16), "fc2bB": fc2bB.astype(np.float32),
    }
    in_maps = []
    for c in range(NCORES):
        m = dict(shared)
        m["x_own"] = np.ascontiguousarray(
            x[:, c * OWN:(c + 1) * OWN, :].reshape(B * OWN, C))
        in_maps.append(m)
    return in_maps


_CACHE = {}


def kernel(**inputs):
    from concourse.bass_utils import run_bass_kernel_spmd

    if "nc" not in _CACHE:
        _CACHE["nc"] = build_kernel()
    nc = _CACHE["nc"]
    in_maps = prepare_inputs(inputs)
    res = run_bass_kernel_spmd(nc, in_maps, core_ids=list(range(NCORES)))
    _CACHE["last_results"] = res

    attn_obj = np.empty((B, 3, N, N), np.float32)
    y = np.empty((B, N, C), np.float32)
    for c in range(NCORES):
        r = res.results[c]
        attn_obj[:, :, c * OWN:(c + 1) * OWN, :] = r["attn_out"]
        y[:, c * OWN:(c + 1) * OWN, :] = r["y_out"].reshape(B, N // NCORES, C)
    return (y, attn_obj)


# revision 20
# speedup vs baseline: 96.1764x; 1.0289x over previous
"""Trainium2 Bass kernel for nn_Block_15006615734251 (dense transformer block
with chained sigmoid/softmax attention), SPMD over 8 NeuronCores.

Sharding: query rows split 8 ways (512 rows/core/batch). Each core computes
the full K/V projections (replicated), its own rows of attn_obj (sigmoid
scores written straight to HBM), its own rows of the obj->pose context chain,
and the epilogue (proj/LN2/MLP) for its own rows. The only cross-core
exchange is an AllGather of ctx1 = sigmoid(S_obj) @ V  ([2,3,4096,64] bf16).
"""

import numpy as np

B, N, C, H = 2, 4096, 384, 6
HD = C // H            # 64
HALF = C // 2          # 192
MLP_H = 4 * C          # 1536
EPS = 1e-5
NCORES = 8
OWN = N // NCORES      # 512 query rows per core per batch
NMT = N // 128         # 32 key tiles per batch
SCALE = HD ** -0.5

# Head pairs: each pair occupies one 128-partition tensor (member0 in
# partitions 0:64, member1 in 64:128). Obj heads (0-2) pair with obj heads,
# pose (3-5) with pose, so the sigmoid and exp phases stay homogeneous.
PAIRS = [
    ((0, 0), (0, 1)),  # obj
    ((1, 0), (1, 1)),  # obj
    ((0, 2), (1, 2)),  # obj (cross-batch)
    ((0, 3), (0, 4)),  # pose
    ((1, 3), (1, 4)),  # pose
    ((0, 5), (1, 5)),  # pose
]
OBJ_PAIRS = [0, 1, 2]
POSE_PAIRS = [3, 4, 5]


def build_kernel():
    import contextlib
    import concourse.bass as bass
    import concourse.tile as tile
    from concourse import bacc, mybir
    from concourse.masks import make_identity
    from concourse.tile import add_dep_helper

    fp32 = mybir.dt.float32
    bf16 = mybir.dt.bfloat16
    AF = mybir.ActivationFunctionType
    ALU = mybir.AluOpType

    nc = bacc.Bacc("TRN2", target_bir_lowering=False, debug=False,
                   enable_asserts=True, num_devices=NCORES)

    # ---- I/O ----
    x_full = nc.dram_tensor("x_full", [B * N, C], fp32, kind="ExternalInput")
    x_own = nc.dram_tensor("x_own", [B * OWN, C], fp32, kind="ExternalInput")
    wT = nc.dram_tensor("wT", [C, 3 * C], bf16, kind="ExternalInput")
    qb_q = nc.dram_tensor("qb_q", [128, 6], fp32, kind="ExternalInput")
    qb_k = nc.dram_tensor("qb_k", [128, 6], fp32, kind="ExternalInput")
    vb = nc.dram_tensor("vb", [128, HALF], fp32, kind="ExternalInput")
    pwT01 = nc.dram_tensor("pwT01", [128, C], bf16, kind="ExternalInput")
    pwT2 = nc.dram_tensor("pwT2", [128, C], bf16, kind="ExternalInput")
    pbB = nc.dram_tensor("pbB", [128, C], fp32, kind="ExternalInput")
    fc1wT = nc.dram_tensor("fc1wT", [C, MLP_H], bf16, kind="ExternalInput")
    fc1b = nc.dram_tensor("fc1b", [128, MLP_H // 128], fp32, kind="ExternalInput")
    fc2wT = nc.dram_tensor("fc2wT", [MLP_H, C], bf16, kind="ExternalInput")
    fc2bB = nc.dram_tensor("fc2bB", [128, C], fp32, kind="ExternalInput")

    attn_out = nc.dram_tensor("attn_out", [B, 3, OWN, N], fp32, kind="ExternalOutput")
    y_out = nc.dram_tensor("y_out", [B * OWN, C], fp32, kind="ExternalOutput")

    with tile.TileContext(nc) as tc, contextlib.ExitStack() as es:
        # ---------------- persistent pools ----------------
        singles = es.enter_context(tc.tile_pool(name="singles", bufs=1))
        persist = es.enter_context(tc.tile_pool(name="persist", bufs=1))
        dram = es.enter_context(tc.tile_pool(name="dram", bufs=1, space="DRAM"))
        psp = es.enter_context(tc.tile_pool(name="psp", bufs=1, space="PSUM"))

        ident = singles.tile([128, 128], bf16)
        make_identity(nc, ident)
        identf = singles.tile([128, 128], fp32)
        make_identity(nc, identf)

        wT_sb = singles.tile([128, 3, 3 * C], bf16)
        nc.sync.dma_start(wT_sb[:], wT.rearrange("(o p) q -> p o q", p=128))
        qbq_sb = singles.tile([128, 6], fp32)
        nc.sync.dma_start(qbq_sb[:], qb_q[:])
        qbk_sb = singles.tile([128, 6], fp32)
        nc.sync.dma_start(qbk_sb[:], qb_k[:])
        vb_sb = singles.tile([128, HALF], fp32)
        nc.sync.dma_start(vb_sb[:], vb[:])
        eps_sb = singles.tile([128, 1], fp32)
        nc.vector.memset(eps_sb[:], EPS)

        x_own_sb = persist.tile([128, B * OWN // 128, C], fp32)
        nc.sync.dma_start(x_own_sb[:], x_own.rearrange("(o p) c -> p o c", p=128))

        kT = [persist.tile([128, N], bf16, name=f"kT{p}") for p in range(6)]
        qT = [persist.tile([128, OWN], bf16, name=f"qT{p}") for p in range(6)]
        v_sb = [persist.tile([128, NMT, HALF], bf16, name=f"v{b}") for b in range(B)]
        c2nT = [persist.tile([128, OWN], bf16, name=f"c2nT{i}") for i in range(3)]

        # ---------------- P0: LN1 -> h^T, then QKV ----------------
        NT_ALL = B * N // 128
        NT_OWN = B * OWN // 128

        with tc.tile_pool(name="hT_pool", bufs=1) as hT_pool, \
             tc.tile_pool(name="p0", bufs=2) as p0:
            hT = hT_pool.tile([128, 3, B * N], bf16)
            hoT = hT_pool.tile([128, 3, B * OWN], bf16)

            def ln_tiles(src_ap, ntiles, dst, group=4):
                for g in range(0, ntiles, group):
                    cnt = min(group, ntiles - g)
                    xg = p0.tile([128, group, C], fp32, tag="xg", bufs=3)
                    nc.sync.dma_start(xg[:, :cnt], src_ap[:, g:g + cnt])
                    st = p0.tile([128, group, 6], fp32, tag="st", bufs=4)
                    mv = p0.tile([128, group, 2], fp32, tag="mv", bufs=4)
                    for j in range(cnt):
                        nc.vector.bn_stats(out=st[:, j], in_=xg[:, j])
                        nc.vector.bn_aggr(out=mv[:, j], in_=st[:, j])
                    rstd = p0.tile([128, group], fp32, tag="rstd", bufs=4)
                    nc.scalar.activation(out=rstd[:, :cnt], in_=mv[:, :cnt, 1],
                                         func=AF.Sqrt, bias=eps_sb[:], scale=1.0)
                    nc.vector.reciprocal(out=rstd[:, :cnt], in_=rstd[:, :cnt])
                    for j in range(cnt):
                        hj = p0.tile([128, C], fp32, tag="hj", bufs=4)
                        nc.vector.tensor_scalar(
                            hj[:], xg[:, j], mv[:, j, 0:1], rstd[:, j:j + 1],
                            ALU.subtract, ALU.mult)
                        for cc in range(3):
                            pt = psp.tile([128, 128], fp32, tag="small", bufs=2,
                                          space="PSUM")
                            nc.tensor.transpose(pt[:], hj[:, cc * 128:(cc + 1) * 128],
                                                identf)
                            nc.vector.tensor_copy(
                                out=dst[:, cc, (g + j) * 128:(g + j + 1) * 128],
                                in_=pt[:])

            ln_tiles(x_full.rearrange("(o p) c -> p o c", p=128), NT_ALL, hT)
            ln_tiles(x_own.rearrange("(o p) c -> p o c", p=128), NT_OWN, hoT)

            def k_col(h):
                return C + h * HD

            def q_col(h):
                return h * HD

            for p, ((bA, hA), (bB_, hB)) in enumerate(PAIRS):
                same_b = (bA == bB_)
                for j in range(N // 512):
                    ps = psp.tile([128, 512], fp32, tag="big", bufs=2, space="PSUM")
                    for cc in range(3):
                        if same_b:
                            nc.tensor.matmul(
                                ps[:], lhsT=wT_sb[:, cc, k_col(hA):k_col(hA) + 128],
                                rhs=hT[:, cc, bA * N + j * 512: bA * N + (j + 1) * 512],
                                start=(cc == 0), stop=(cc == 2))
                        else:
                            nc.tensor.matmul(
                                ps[0:64], lhsT=wT_sb[:, cc, k_col(hA):k_col(hA) + 64],
                                rhs=hT[:, cc, bA * N + j * 512: bA * N + (j + 1) * 512],
                                start=(cc == 0), stop=(cc == 2), tile_position=(0, 0))
                            nc.tensor.matmul(
                                ps[64:128], lhsT=wT_sb[:, cc, k_col(hB):k_col(hB) + 64],
                                rhs=hT[:, cc, bB_ * N + j * 512: bB_ * N + (j + 1) * 512],
                                start=(cc == 0), stop=(cc == 2), tile_position=(0, 64))
                    nc.vector.tensor_scalar(
                        kT[p][:, j * 512:(j + 1) * 512], ps[:],
                        qbk_sb[:, p:p + 1], None, ALU.add)
                ps = psp.tile([128, 512], fp32, tag="big", bufs=2, space="PSUM")
                for cc in range(3):
                    if same_b:
                        nc.tensor.matmul(
                            ps[:], lhsT=wT_sb[:, cc, q_col(hA):q_col(hA) + 128],
                            rhs=hoT[:, cc, bA * OWN:(bA + 1) * OWN],
                            start=(cc == 0), stop=(cc == 2))
                    else:
                        nc.tensor.matmul(
                            ps[0:64], lhsT=wT_sb[:, cc, q_col(hA):q_col(hA) + 64],
                            rhs=hoT[:, cc, bA * OWN:(bA + 1) * OWN],
                            start=(cc == 0), stop=(cc == 2), tile_position=(0, 0))
                        nc.tensor.matmul(
                            ps[64:128], lhsT=wT_sb[:, cc, q_col(hB):q_col(hB) + 64],
                            rhs=hoT[:, cc, bB_ * OWN:(bB_ + 1) * OWN],
                            start=(cc == 0), stop=(cc == 2), tile_position=(0, 64))
                nc.vector.tensor_scalar(
                    qT[p][:], ps[:], qbq_sb[:, p:p + 1], None, ALU.add)

            for b in range(B):
                for mt in range(NMT):
                    ps = psp.tile([128, HALF], fp32, tag="small", bufs=2, space="PSUM")
                    for cc in range(3):
                        nc.tensor.matmul(
                            ps[:],
                            lhsT=hT[:, cc, b * N + mt * 128: b * N + (mt + 1) * 128],
                            rhs=wT_sb[:, cc, 2 * C + HALF: 3 * C],
                            start=(cc == 0), stop=(cc == 2))
                    nc.vector.tensor_tensor(v_sb[b][:, mt], ps[:], vb_sb[:], ALU.add)

        # ---------------- P1a: obj chains (sigmoid -> ctx1 -> gather) ----------
        gath = [dram.tile([NCORES, 2, OWN, HD], bf16, addr_space="Shared",
                          name=f"gath{i}") for i in range(3)]
        loc = [dram.tile([2, OWN, HD], bf16, name=f"loc{i}") for i in range(3)]

        with tc.tile_pool(name="p1", bufs=3) as p1, \
             tc.tile_pool(name="p2", bufs=3) as p2, \
             tc.tile_pool(name="c1pool", bufs=1) as c1pool:
            ctx1e = [c1pool.tile([128, 2, NMT, 66], bf16, name=f"ctx1e{i}")
                     for i in range(3)]
            for p in OBJ_PAIRS:
                (bA, hA), (bB_, hB) = PAIRS[p]
                iA, iB = hA, hB
                ps_c1 = psp.tile([128, 512], fp32, tag="accum", bufs=2, space="PSUM")
                for mt in range(NMT):
                    ps = psp.tile([128, 2 * OWN], fp32, tag="big", bufs=2, space="PSUM")
                    nc.tensor.matmul(
                        ps[:, 0:OWN], lhsT=kT[p][0:64, mt * 128:(mt + 1) * 128],
                        rhs=qT[p][0:64], tile_position=(0, 0))
                    nc.tensor.matmul(
                        ps[:, OWN:], lhsT=kT[p][64:128, mt * 128:(mt + 1) * 128],
                        rhs=qT[p][64:128], tile_position=(64, 0))
                    at2 = p1.tile([128, 2 * OWN], bf16, tag="at2", bufs=4)
                    nc.scalar.activation(out=at2[:], in_=ps[:], func=AF.Sigmoid)
                    nc.tensor.matmul(
                        ps_c1[0:64], lhsT=v_sb[bA][:, mt, iA * HD:(iA + 1) * HD],
                        rhs=at2[:, 0:OWN], start=(mt == 0), stop=(mt == NMT - 1),
                        tile_position=(0, 0), skip_group_check=True)
                    nc.tensor.matmul(
                        ps_c1[64:128], lhsT=v_sb[bB_][:, mt, iB * HD:(iB + 1) * HD],
                        rhs=at2[:, OWN:], start=(mt == 0), stop=(mt == NMT - 1),
                        tile_position=(0, 64), skip_group_check=True)
                c1sb = p1.tile([128, 512], bf16, tag="c1sb")
                nc.vector.tensor_copy(out=c1sb[:], in_=ps_c1[:])
                c1row = c1pool.tile([128, 2, OWN // 128, HD], bf16, tag=f"c1row{p}")
                for qc in range(OWN // 128):
                    pt = psp.tile([128, 128], bf16, tag="small", bufs=2, space="PSUM")
                    nc.tensor.transpose(pt[:], c1sb[:, qc * 128:(qc + 1) * 128], ident)
                    nc.vector.tensor_copy(out=c1row[:, 0, qc], in_=pt[:, 0:64])
                    nc.vector.tensor_copy(out=c1row[:, 1, qc], in_=pt[:, 64:128])
                nc.sync.dma_start(
                    loc[p].rearrange("m (q pp) d -> pp m q d", pp=128), c1row[:])
                nc.gpsimd.collective_compute(
                    "AllGather", ALU.bypass,
                    replica_groups=[list(range(NCORES))],
                    ins=[loc[p].opt()], outs=[gath[p].opt()])
                for r in range(NCORES):
                    for m in range(2):
                        nc.sync.dma_start(
                            ctx1e[p][:, m, r * (OWN // 128):(r + 1) * (OWN // 128), 0:HD],
                            gath[p][r, m].rearrange("(q pp) d -> pp q d", pp=128))
                nc.vector.memset(ctx1e[p][:, :, :, HD:HD + 2], 1.0)

            # ---------------- P2: pose (exp softmax) + ctx2 ----------------
            ones1 = c1pool.tile([1, 64], fp32)
            nc.vector.memset(ones1[:], 1.0)
            for p in POSE_PAIRS:
                (bA, hA), (bB_, hB) = PAIRS[p]
                op = p - 3  # matching obj pair index / ctx1e index
                ps_c2a = psp.tile([65, OWN], fp32, tag="accum", bufs=2, space="PSUM")
                ps_c2b = psp.tile([65, OWN], fp32, tag="accum", bufs=2, space="PSUM")
                for mt in range(NMT):
                    ps = psp.tile([128, 2 * OWN], fp32, tag="big", bufs=2, space="PSUM")
                    nc.tensor.matmul(
                        ps[:, 0:OWN], lhsT=kT[p][0:64, mt * 128:(mt + 1) * 128],
                        rhs=qT[p][0:64], tile_position=(0, 0))
                    nc.tensor.matmul(
                        ps[:, OWN:], lhsT=kT[p][64:128, mt * 128:(mt + 1) * 128],
                        rhs=qT[p][64:128], tile_position=(64, 0))
                    et2 = p2.tile([128, 2 * OWN], bf16, tag="et2", bufs=5)
                    nc.scalar.activation(out=et2[:], in_=ps[:], func=AF.Exp)
                    nc.tensor.matmul(
                        ps_c2a[:], lhsT=ctx1e[op][:, 0, mt, 0:65],
                        rhs=et2[:, 0:OWN], start=(mt == 0), stop=(mt == NMT - 1),
                        skip_group_check=True)
                    nc.tensor.matmul(
                        ps_c2b[:], lhsT=ctx1e[op][:, 1, mt, 0:65],
                        rhs=et2[:, OWN:], start=(mt == 0), stop=(mt == NMT - 1),
                        skip_group_check=True)
                for half, psx in ((0, ps_c2a), (1, ps_c2b)):
                    den = p2.tile([1, OWN], fp32, tag="den")
                    nc.vector.tensor_copy(out=den[:], in_=psx[64:65, :])
                    nc.vector.reciprocal(out=den[:], in_=den[:])
                    psb = psp.tile([64, OWN], fp32, tag="small", bufs=2, space="PSUM")
                    nc.tensor.matmul(psb[:], lhsT=ones1[:], rhs=den[:])
                    c2u = p2.tile([64, OWN], fp32, tag="c2u")
                    nc.vector.tensor_copy(out=c2u[:], in_=psx[0:64, :])
                    nc.vector.tensor_tensor(
                        c2nT[op][64 * half:64 * half + 64, :],
                        c2u[:], psb[:], ALU.mult)

            # ---------------- P1b: obj row-major scores -> attn_obj ----------
            for p in OBJ_PAIRS:
                (bA, hA), (bB_, hB) = PAIRS[p]
                iA, iB = hA, hB
                for half in range(2):
                    base = 64 * half
                    b_, i_ = (bA, iA) if half == 0 else (bB_, iB)
                    for nt in range(OWN // 128):
                        for mh in range(N // 1024):
                            ps = psp.tile([128, 2 * OWN], fp32, tag="big", bufs=2,
                                          space="PSUM")
                            for mc in range(2):
                                nc.tensor.matmul(
                                    ps[:, mc * 512:(mc + 1) * 512],
                                    lhsT=qT[p][base:base + 64, nt * 128:(nt + 1) * 128],
                                    rhs=kT[p][base:base + 64,
                                              mh * 1024 + mc * 512:
                                              mh * 1024 + (mc + 1) * 512],
                                    tile_position=(base, 0))
                            stg = p1.tile([128, 1024], fp32, tag="stg", bufs=4)
                            nc.scalar.activation(out=stg[:], in_=ps[:], func=AF.Sigmoid)
                            nc.sync.dma_start(
                                attn_out[b_, i_, nt * 128:(nt + 1) * 128,
                                         mh * 1024:(mh + 1) * 1024],
                                stg[:])

        # ---------------- P3: proj + LN2 + MLP ----------------
        with tc.tile_pool(name="p3", bufs=3) as p3, \
             tc.tile_pool(name="p3s", bufs=1) as p3s:
            pw01_sb = p3s.tile([128, C], bf16)
            nc.sync.dma_start(pw01_sb[:], pwT01[:])
            pw2_sb = p3s.tile([128, C], bf16)
            nc.sync.dma_start(pw2_sb[:], pwT2[:])
            pbB_sb = p3s.tile([128, C], fp32)
            nc.sync.dma_start(pbB_sb[:], pbB[:])
            fc1w_sb = p3s.tile([128, 3, MLP_H], bf16)
            nc.sync.dma_start(fc1w_sb[:], fc1wT.rearrange("(o p) q -> p o q", p=128))
            fc1b_sb = p3s.tile([128, MLP_H // 128], fp32)
            nc.sync.dma_start(fc1b_sb[:], fc1b[:])
            fc2w_sb = p3s.tile([128, MLP_H // 128, C], bf16)
            nc.sync.dma_start(fc2w_sb[:], fc2wT.rearrange("(o p) c -> p o c", p=128))
            fc2b_sb = p3s.tile([128, C], fp32)
            nc.sync.dma_start(fc2b_sb[:], fc2bB[:])

            x2_sb = p3s.tile([128, B * OWN // 128, C], fp32)
            h2T = p3s.tile([128, 3, B * OWN], bf16)
            c2_of_b = {0: c2nT[0], 1: c2nT[1]}
            for b in range(B):
                for qc in range(OWN // 128):
                    ps = psp.tile([128, C], fp32, tag="big", bufs=2, space="PSUM")
                    nc.tensor.matmul(
                        ps[:], lhsT=c2_of_b[b][:, qc * 128:(qc + 1) * 128],
                        rhs=pw01_sb[:], start=True, stop=False,
                        skip_group_check=True)
                    base = 64 * b
                    nc.tensor.matmul(
                        ps[:], lhsT=c2nT[2][base:base + 64, qc * 128:(qc + 1) * 128],
                        rhs=pw2_sb[base:base + 64], start=False, stop=True,
                        skip_group_check=True)
                    ti = b * (OWN // 128) + qc
                    xt = x2_sb[:, ti]
                    nc.vector.tensor_tensor(xt, ps[:], x_own_sb[:, ti], ALU.add)
                    nc.vector.tensor_tensor(xt, xt, pbB_sb[:], ALU.add)
                    st = p3.tile([128, 6], fp32, tag="st2")
                    mv = p3.tile([128, 2], fp32, tag="mv2")
                    nc.vector.bn_stats(out=st[:], in_=xt)
                    nc.vector.bn_aggr(out=mv[:], in_=st[:])
                    rstd = p3.tile([128, 1], fp32, tag="rstd2")
                    nc.scalar.activation(out=rstd[:], in_=mv[:, 1:2], func=AF.Sqrt,
                                         bias=eps_sb[:], scale=1.0)
                    nc.vector.reciprocal(out=rstd[:], in_=rstd[:])
                    h2 = p3.tile([128, C], bf16, tag="h2")
                    nc.vector.tensor_scalar(
                        h2[:], xt, mv[:, 0:1], rstd[:], ALU.subtract, ALU.mult)
                    for cc in range(3):
                        pt = psp.tile([128, 128], bf16, tag="small", bufs=2,
                                      space="PSUM")
                        nc.tensor.transpose(pt[:], h2[:, cc * 128:(cc + 1) * 128], ident)
                        nc.vector.tensor_copy(
                            out=h2T[:, cc, ti * 128:(ti + 1) * 128], in_=pt[:])

            g_sb = p3s.tile([128, MLP_H // 128, B * OWN], bf16)
            for s in range(MLP_H // 128):
                for qh in range(B * OWN // 512):
                    ps = psp.tile([128, 512], fp32, tag="big", bufs=2, space="PSUM")
                    for cc in range(3):
                        nc.tensor.matmul(
                            ps[:], lhsT=fc1w_sb[:, cc, s * 128:(s + 1) * 128],
                            rhs=h2T[:, cc, qh * 512:(qh + 1) * 512],
                            start=(cc == 0), stop=(cc == 2))
                    nc.scalar.activation(
                        out=g_sb[:, s, qh * 512:(qh + 1) * 512], in_=ps[:],
                        func=AF.Gelu, bias=fc1b_sb[:, s:s + 1], scale=1.0)

            for ti in range(B * OWN // 128):
                ps = psp.tile([128, C], fp32, tag="big", bufs=2, space="PSUM")
                for s in range(MLP_H // 128):
                    nc.tensor.matmul(
                        ps[:], lhsT=g_sb[:, s, ti * 128:(ti + 1) * 128],
                        rhs=fc2w_sb[:, s], start=(s == 0),
                        stop=(s == MLP_H // 128 - 1))
                yt = p3.tile([128, C], fp32, tag="yt")
                nc.vector.tensor_tensor(yt[:], ps[:], x2_sb[:, ti], ALU.add)
                nc.vector.tensor_tensor(yt[:], yt[:], fc2b_sb[:], ALU.add)
                nc.sync.dma_start(
                    y_out.rearrange("(o p) c -> p o c", p=128)[:, ti], yt[:])

    nc.finalize()
    return nc


def prepare_inputs(inputs):
    """Host-side: fold gammas/betas/score-scale into weights, build per-core maps."""
    import ml_dtypes
    bf16 = ml_dtypes.bfloat16

    x = np.asarray(inputs["x"], np.float32)
    qkv_w = np.asarray(inputs["qkv_w"], np.float32)
    proj_w = np.asarray(inputs["proj_w"], np.float32)
    proj_b = np.asarray(inputs["proj_b"], np.float32)
    g1 = np.asarray(inputs["gamma1"], np.float32)
    b1 = np.asarray(inputs["beta1"], np.float32)
    g2 = np.asarray(inputs["gamma2"], np.float32)
    b2 = np.asarray(inputs["beta2"], np.float32)
    fc1_w = np.asarray(inputs["fc1_w"], np.float32)
    fc1_b = np.asarray(inputs["fc1_b"], np.float32)
    fc2_w = np.asarray(inputs["fc2_w"], np.float32)
    fc2_b = np.asarray(inputs["fc2_b"], np.float32)

    wp = qkv_w * g1[None, :]
    qkv_bias = qkv_w @ b1
    wp[:C] *= SCALE
    qkv_bias[:C] *= SCALE
    wT = np.ascontiguousarray(wp.T)

    def pair_bias(sec):
        out = np.zeros((128, 6), np.float32)
        for p, ((bA, hA), (bB_, hB)) in enumerate(PAIRS):
            out[0:64, p] = qkv_bias[sec + hA * HD: sec + (hA + 1) * HD]
            out[64:128, p] = qkv_bias[sec + hB * HD: sec + (hB + 1) * HD]
        return out

    qb_q = pair_bias(0)
    qb_k = pair_bias(C)
    vb = np.broadcast_to(qkv_bias[2 * C + HALF:], (128, HALF)).copy()

    pwT = np.ascontiguousarray(proj_w.T)
    pwT01 = pwT[0:128]
    pwT2 = np.concatenate([pwT[128:192], pwT[128:192]], axis=0)
    pbB = np.broadcast_to(proj_b, (128, C)).copy()

    fc1wp = fc1_w * g2[None, :]
    fc1bp = fc1_b + fc1_w @ b2
    fc1wT = np.ascontiguousarray(fc1wp.T)
    fc1b_t = np.ascontiguousarray(fc1bp.reshape(MLP_H // 128, 128).T)
    fc2wT = np.ascontiguousarray(fc2_w.T)
    fc2bB = np.broadcast_to(fc2_b, (128, C)).copy()

    shared = {
        "x_full": np.ascontiguousarray(x.reshape(B * N, C)),
        "wT": wT.astype(bf16),
        "qb_q": qb_q, "qb_k": qb_k, "vb": vb,
        "pwT01": pwT01.astype(bf16), "pwT2": pwT2.astype(bf16),
        "pbB": pbB.astype(np.float32),
        "fc1wT": fc1wT.astype(bf16), "fc1b": fc1b_t.astype(np.float32),
        "fc2wT": fc2wT.astype(bf16), "fc2bB": fc2bB.astype(np.float32),
    }
    in_maps = []
    for c in range(NCORES):
        m = dict(shared)
        m["x_own"] = np.ascontiguousarray(
            x[:, c * OWN:(c + 1) * OWN, :].reshape(B * OWN, C))
        in_maps.append(m)
    return in_maps


_CACHE = {}


def kernel(**inputs):
    from concourse.bass_utils import run_bass_kernel_spmd

    if "nc" not in _CACHE:
        _CACHE["nc"] = build_kernel()
    nc = _CACHE["nc"]
    in_maps = prepare_inputs(inputs)
    res = run_bass_kernel_spmd(nc, in_maps, core_ids=list(range(NCORES)))
    _CACHE["last_results"] = res

    attn_obj = np.empty((B, 3, N, N), np.float32)
    y = np.empty((B, N, C), np.float32)
    for c in range(NCORES):
        r = res.results[c]
        attn_obj[:, :, c * OWN:(c + 1) * OWN, :] = r["attn_out"]
        y[:, c * OWN:(c + 1) * OWN, :] = r["y_out"].reshape(B, N // NCORES, C)
    return (y, attn_obj)
